# revision 1
# baseline (speedup 1.0000x reference)
"""Distributed sparse-MoE routing kernel for 8 Trainium2 NeuronCores.

Algorithm notes
---------------
The reference module routes T=16384 tokens (top-1 of E=8 experts, capacity
C=100, tokens past capacity dropped) and applies ONE shared expert weight
(H -> H Linear) to the dispatched slots.  Because the expert weight is shared,
the output collapses to

    out[t] = gate_t * (x_t @ W + b)   if token t wins a capacity slot
           = 0                        otherwise

so only <= E*C = 800 of 16384 tokens need the big matmul.  Token t (choosing
expert e) wins a slot iff fewer than C earlier tokens (global token order)
chose e.

Distribution: tokens are sharded over the 8 cores in 32-token blocks, strided
(core k owns blocks b with b % 8 == k).  This balances the winning tokens
(~100/core) while keeping the global running count decomposable: each core
computes per-block expert counts locally, one small AllGather (64x8 floats per
core) shares them, and a couple of small matmuls turn the gathered table into
per-block base offsets.  Everything else (router matmul, softmax/argmax,
capacity cumsum, stream compaction, gather, expert matmul, scatter) is local
to each core.

Measured constraints on this fleet (do not re-derive):
- A collective costs ~60-85us in an empty kernel; amortized here to ~30us
  (~15us cross-core launch skew + ~15us AllGather).  AllToAll emulation and
  pairwise splits both measured WORSE despite better documented floors.
- The router must run in full fp32: min top-2 logit gap on the seed-0 data
  is 1.39e-05 absolute, while f32r matmul error measures ~1.5e-4 relative
  (so f32r/bf16 routing flips argmax vs the reference).  fp32 moving-operand
  streams at 4 cycles/row on the PE -- the dominant phase-A cost.
- Splitting DMAs across the SP+ACT HWDGE engines measured ~25us WORSE (the
  ACT instruction stream serializes triggers with the softmax Exp work).
- PE executes in program order: AG-independent matmuls must be EMITTED
  before AG-dependent ones to fill the collective window (done below).
- exec_time ~131us median, ~121us best; run-to-run variance +-8us plus rare
  ~2x fleet outliers and a ~10% transient NRT error (retried in kernel()).
"""
import os
import sys
import types
from contextlib import ExitStack

sys.path.insert(0, "/opt/trn_rl_repo")

import numpy as np

import concourse.bass as bass
import concourse.bacc as bacc
import concourse.mybir as mybir
import concourse.tile as tile
from concourse import bass_utils

F32 = mybir.dt.float32
F32R = mybir.dt.float32r
I32 = mybir.dt.int32
AX = mybir.AxisListType
ALU = mybir.AluOpType
ACT = mybir.ActivationFunctionType

P = 128          # SBUF partitions / tile rows
H = 1024         # hidden dim
E = 8            # experts
C = 100          # capacity
NCORES = 8
T_LOC = 2048     # tokens per core
NTILE = T_LOC // P   # 16 token tiles per core
NCH = H // P         # 8 hidden chunks
BLK = 32             # token block size for sharding
NBLK = T_LOC // BLK  # 64 local blocks per core
KMAX = 128           # max compacted (kept) tokens per core (<=114 actual)
BIG = 4096.0         # scatter-index offset used to mark dropped slots


def _expert_dtype():
    return F32 if os.environ.get("MOE_EXPERT_F32") else F32R


def build():
    """Build + compile the SPMD program (identical on all 8 cores)."""
    nc = bacc.Bacc("TRN2", target_bir_lowering=False, debug=False,
                   num_devices=NCORES)

    x = nc.dram_tensor("x", [T_LOC, H], F32, kind="ExternalInput")
    xt = nc.dram_tensor("xt", [H, T_LOC], F32, kind="ExternalInput")
    wg = nc.dram_tensor("w_gate", [H, E], F32, kind="ExternalInput")
    we = nc.dram_tensor("w_expert", [H, H], _expert_dtype(), kind="ExternalInput")
    be = nc.dram_tensor("b_expert", [1, H], _expert_dtype(), kind="ExternalInput")
    # constants (host-computed)
    tri = nc.dram_tensor("tri128", [P, P], F32, kind="ExternalInput")
    ident = nc.dram_tensor("ident", [P, P], F32, kind="ExternalInput")
    iota = nc.dram_tensor("iota256", [P, KMAX], F32, kind="ExternalInput")
    tidx = nc.dram_tensor("tidx16", [P, NTILE], F32, kind="ExternalInput")
    esum = nc.dram_tensor("esum", [P, 4], F32, kind="ExternalInput")
    eexp = nc.dram_tensor("eexp", [4, P], F32, kind="ExternalInput")
    ones1 = nc.dram_tensor("ones1", [1, P], F32, kind="ExternalInput")
    onescol = nc.dram_tensor("onescol", [P, 1], F32, kind="ExternalInput")
    neg1 = nc.dram_tensor("neg1", [1, E], F32, kind="ExternalInput")
    h64 = nc.dram_tensor("h64", [NBLK, NBLK], F32, kind="ExternalInput")
    maskk = nc.dram_tensor("maskk", [NCORES * NBLK, NBLK], F32,
                           kind="ExternalInput")
    tri16 = nc.dram_tensor("tri16", [NTILE, NTILE], F32, kind="ExternalInput")

    out = nc.dram_tensor("out", [T_LOC, H], F32, kind="ExternalOutput")

    with tile.TileContext(nc) as tc:
        _body(nc, tc, x, xt, wg, we, be, tri, ident, iota, tidx, esum, eexp,
              ones1, onescol, neg1, h64, maskk, tri16, out)

    nc.compile()
    return nc


def _body(nc, tc, x, xt, wg, we, be, tri, ident, iota, tidx, esum, eexp,
          ones1, onescol, neg1, h64, maskk, tri16, out):
    EDT = _expert_dtype()
    with ExitStack() as top:
        sb = top.enter_context(tc.tile_pool(name="sb", bufs=1))
        st = top.enter_context(tc.tile_pool(name="st", bufs=4))
        dram = top.enter_context(tc.tile_pool(name="dram", bufs=1, space="DRAM"))

        # ---- router weights + identity first: they gate every matmul ---
        wg_sb = sb.tile([P, NCH * E], F32, tag="wg")
        nc.sync.dma_start(wg_sb[:].rearrange("p (c e) -> p c e", c=NCH),
                          wg[:, :].rearrange("(c p) e -> p c e", p=P))
        ident_sb = sb.tile([P, P], F32, tag="ident")
        nc.sync.dma_start(ident_sb[:], ident[:, :])
        # x^T next, group-major so router group 0 starts early
        xTf = sb.tile([P, NCH * T_LOC], F32, tag="xTf")
        for g in range(4):
            for c in range(NCH):
                nc.sync.dma_start(
                    xTf[:, c * T_LOC + g * 512:c * T_LOC + (g + 1) * 512],
                    xt[c * P:(c + 1) * P, g * 512:(g + 1) * 512])

        # ---- constant loads --------------------------------------------
        tri_sb = sb.tile([P, P], F32, tag="tri")
        nc.sync.dma_start(tri_sb[:], tri[:, :])
        iota_sb = sb.tile([P, KMAX], F32, tag="iota")
        nc.sync.dma_start(iota_sb[:], iota[:, :])
        tidx_sb = sb.tile([P, NTILE], F32, tag="tidx")
        nc.sync.dma_start(tidx_sb[:], tidx[:, :])
        esum_sb = sb.tile([P, 4], F32, tag="esum")
        nc.sync.dma_start(esum_sb[:], esum[:, :])
        eexp_sb = sb.tile([4, P], F32, tag="eexp")
        nc.sync.dma_start(eexp_sb[:], eexp[:, :])
        ones1_sb = sb.tile([1, P], F32, tag="ones1")
        nc.sync.dma_start(ones1_sb[:], ones1[:, :])
        onescol_sb = sb.tile([P, 1], F32, tag="onescol")
        nc.sync.dma_start(onescol_sb[:], onescol[:, :])
        neg1_sb = sb.tile([1, E], F32, tag="neg1")
        nc.sync.dma_start(neg1_sb[:], neg1[:, :])
        h64_sb = sb.tile([NBLK, NBLK], F32, tag="h64")
        nc.sync.dma_start(h64_sb[:], h64[:, :])
        tri16_sb = sb.tile([NTILE, NTILE], F32, tag="tri16")
        nc.sync.dma_start(tri16_sb[:], tri16[:, :])
        # maskk [512, 64] -> [128, 4, 64]
        maskk_sb = sb.tile([P, 4 * NBLK], F32, tag="maskk")
        nc.sync.dma_start(maskk_sb[:].rearrange("p (c j) -> p c j", c=4),
                          maskk[:, :].rearrange("(c p) j -> p c j", p=P))
        # ---- persistent per-token state --------------------------------
        masks_sb = sb.tile([P, NTILE * E], F32, tag="masks")
        gate_sb = sb.tile([P, NTILE], F32, tag="gate")
        s_sb = sb.tile([P, NTILE], F32, tag="s")
        kf_sb = sb.tile([P, NTILE], F32, tag="kf")
        bc_sb = sb.tile([4, NTILE * E], F32, tag="bc")   # per-block counts
        tks_sb = sb.tile([1, NTILE], F32, tag="tks")     # per-tile kept counts

        # ================= PHASE A: router + masks + counts =============
        # x^T comes pre-transposed from the host (pure layout prep); the
        # router streams it through the PE with w_gate stationary, then the
        # [8, T] logits are transposed back in cheap [8,128] chunks.
        logits_sb = sb.tile([P, NTILE * E], F32, tag="logits")
        with ExitStack() as pa:
            pbig = pa.enter_context(tc.tile_pool(name="pbig", bufs=3, space="PSUM"))
            psml = pa.enter_context(tc.tile_pool(name="psml", bufs=4, space="PSUM"))

            for g in range(4):          # 4 groups of 512 tokens
                lgT = pbig.tile([E, 512], F32, space="PSUM", tag="lgT")
                for c in range(NCH):
                    nc.tensor.matmul(
                        lgT[:], lhsT=wg_sb[:, c * E:(c + 1) * E],
                        rhs=xTf[:, c * T_LOC + g * 512: c * T_LOC + (g + 1) * 512],
                        start=(c == 0), stop=(c == NCH - 1))
                lgs = st.tile([E, 512], F32, tag="lgs")
                nc.vector.tensor_copy(lgs[:], lgT[:])
                for j in range(4):      # transpose back per 128-token tile
                    i = g * 4 + j
                    ltp = psml.tile([P, E], F32, space="PSUM", tag="sm")
                    nc.tensor.transpose(ltp[:], lgs[:, j * P:(j + 1) * P],
                                        ident_sb[:E, :E])
                    nc.vector.tensor_copy(logits_sb[:, i * E:(i + 1) * E], ltp[:])

            for g in range(4):          # batched softmax / first-max mask
                l32 = logits_sb[:, 32 * g:32 * (g + 1)]
                l3d = l32.rearrange("p (t e) -> p t e", e=E)
                m4 = st.tile([P, 4], F32, tag="m4")
                nc.vector.reduce_max(m4[:], l3d, axis=AX.X)
                m4b = m4[:].rearrange("p (t o) -> p t o", o=1).to_broadcast(
                    [P, 4, E])
                d32 = st.tile([P, 32], F32, tag="d32")
                nc.vector.tensor_tensor(
                    d32[:].rearrange("p (t e) -> p t e", e=E), l3d, m4b,
                    op=ALU.subtract)
                e32 = st.tile([P, 32], F32, tag="e32")
                nc.scalar.activation(e32[:], d32[:], ACT.Exp)
                z4 = st.tile([P, 4], F32, tag="z4")
                nc.vector.reduce_sum(
                    z4[:], e32[:].rearrange("p (t e) -> p t e", e=E), axis=AX.X)
                nc.vector.reciprocal(gate_sb[:, 4 * g:4 * (g + 1)], z4[:])
                mraw = st.tile([P, 32], F32, tag="mraw32")
                nc.vector.tensor_tensor(
                    mraw[:].rearrange("p (t e) -> p t e", e=E), l3d, m4b,
                    op=ALU.is_equal)
                c1 = mraw
                for sh in (1, 2, 4):
                    c2 = st.tile([P, 32], F32, tag=f"cc{sh}")
                    c1v = c1[:].rearrange("p (t e) -> p t e", e=E)
                    c2v = c2[:].rearrange("p (t e) -> p t e", e=E)
                    nc.vector.tensor_copy(c2v[:, :, :sh], c1v[:, :, :sh])
                    nc.vector.tensor_tensor(c2v[:, :, sh:], c1v[:, :, sh:],
                                            c1v[:, :, :E - sh], op=ALU.add)
                    c1 = c2
                mk32 = masks_sb[:, 32 * g:32 * (g + 1)]
                nc.vector.tensor_scalar(mk32, c1[:], 1.0, None,
                                        op0=ALU.is_equal)
                nc.vector.tensor_tensor(mk32, mk32, mraw[:], op=ALU.mult)

            for g in range(4):          # per-block counts, 4 tiles per matmul
                bcp = psml.tile([4, 32], F32, space="PSUM", tag="sm")
                nc.tensor.matmul(bcp[:], lhsT=esum_sb[:],
                                 rhs=masks_sb[:, 32 * g:32 * (g + 1)],
                                 start=True, stop=True)
                nc.vector.tensor_copy(bc_sb[:, 32 * g:32 * (g + 1)], bcp[:])

        # ================= AllGather of per-block counts ================
        agin = dram.tile([NBLK, E], F32, tag="agin")
        agout = dram.tile([NCORES * NBLK, E], F32, tag="agout")
        scr = dram.tile([NBLK, E], F32, tag="scr")
        # bc_sb [4, (i e)] -> dram rows j = 4i+q
        nc.sync.dma_start(agin[:].rearrange("(i q) e -> q i e", q=4),
                           bc_sb[:].rearrange("p (i e) -> p i e", e=E))
        nc.gpsimd.collective_compute(
            "AllGather", ALU.bypass,
            replica_groups=[list(range(NCORES))],
            ins=[agin[:].opt()], outs=[agout[:].opt()])
        # gathered table -> [128, 4, 8]
        agt_sb = sb.tile([P, 4 * E], F32, tag="agt")
        nc.sync.dma_start(agt_sb[:].rearrange("p (c e) -> p c e", c=4),
                          agout[:].rearrange("(c p) e -> p c e", p=P))
        # my own counts as [64, 8]
        bc64_sb = sb.tile([NBLK, E], F32, tag="bc64")
        nc.sync.dma_start(bc64_sb[:], agin[:])
        # expert weights land during the AllGather wait + phase B (they are
        # first read in phase C, so keep them off phase A's DMA bandwidth)
        we_sb = sb.tile([P, NCH * H], EDT, tag="we")
        for c in range(NCH):
            nc.sync.dma_start(we_sb[:, c * H:(c + 1) * H],
                              we[c * P:(c + 1) * P, :])
        be_sb = sb.tile([1, H], EDT, tag="be")
        nc.sync.dma_start(be_sb[:], be[:, :])

        with ExitStack() as pb:
            psml = pb.enter_context(tc.tile_pool(name="psml2", bufs=2, space="PSUM"))
            ploc = pb.enter_context(tc.tile_pool(name="ploc", bufs=1, space="PSUM"))
            pcmp = pb.enter_context(tc.tile_pool(name="pcmp", bufs=1, space="PSUM"))

            # PE executes in program order, so emit every AG-independent
            # matmul FIRST: the local cumsum partials and the own-counts part
            # of the base fill the PE while the collective is in flight.
            loc4s = []
            for g in range(4):
                loc4 = ploc.tile([P, 32], F32, space="PSUM", tag=f"loc{g}")
                nc.tensor.matmul(loc4[:], lhsT=tri_sb[:],
                                 rhs=masks_sb[:, 32 * g:32 * (g + 1)],
                                 start=True, stop=False)
                loc4s.append(loc4)
            # addbase[j, e] = own earlier-tile base - 1 + cross-core base
            ab = psml.tile([NBLK, E], F32, space="PSUM", tag="sm")
            nc.tensor.matmul(ab[:], lhsT=h64_sb[:], rhs=bc64_sb[:],
                             start=True, stop=False)
            nc.tensor.matmul(ab[:], lhsT=ones1_sb[:, :NBLK], rhs=neg1_sb[:],
                             start=False, stop=False)
            for c in range(4):
                nc.tensor.matmul(ab[:], lhsT=maskk_sb[:, c * NBLK:(c + 1) * NBLK],
                                 rhs=agt_sb[:, c * E:(c + 1) * E],
                                 start=False, stop=(c == 3))
            ab_sb = sb.tile([NBLK, E], F32, tag="ab64")
            nc.vector.tensor_copy(ab_sb[:], ab[:])
            # reorder to [4, 16*8] via DRAM bounce
            nc.sync.dma_start(scr[:], ab_sb[:])
            addbase_sb = sb.tile([4, NTILE * E], F32, tag="addbase")
            nc.sync.dma_start(addbase_sb[:].rearrange("p (i e) -> p i e", e=E),
                              scr[:].rearrange("(i q) e -> q i e", q=4))

            # ============== PHASE B: keep / gate-scale / kept flags =====
            for g in range(4):
                loc4 = loc4s[g]
                nc.tensor.matmul(loc4[:], lhsT=eexp_sb[:],
                                 rhs=addbase_sb[:, 32 * g:32 * (g + 1)],
                                 start=False, stop=True)
                keep32 = st.tile([P, 32], F32, tag="keep32")
                nc.vector.tensor_scalar(keep32[:], loc4[:], float(C), None,
                                        op0=ALU.is_lt)
                nc.vector.tensor_tensor(keep32[:], keep32[:],
                                        masks_sb[:, 32 * g:32 * (g + 1)],
                                        op=ALU.mult)
                nc.vector.reduce_sum(
                    kf_sb[:, 4 * g:4 * (g + 1)],
                    keep32[:].rearrange("p (t e) -> p t e", e=E), axis=AX.X)
                s32 = st.tile([P, 32], F32, tag="s32")
                g4b = gate_sb[:, 4 * g:4 * (g + 1)].rearrange(
                    "p (t o) -> p t o", o=1).to_broadcast([P, 4, E])
                nc.vector.tensor_tensor(
                    s32[:].rearrange("p (t e) -> p t e", e=E),
                    keep32[:].rearrange("p (t e) -> p t e", e=E), g4b,
                    op=ALU.mult)
                nc.vector.reduce_sum(
                    s_sb[:, 4 * g:4 * (g + 1)],
                    s32[:].rearrange("p (t e) -> p t e", e=E), axis=AX.X)
                tkp = psml.tile([1, 4], F32, space="PSUM", tag="sm")
                nc.tensor.matmul(tkp[:], lhsT=onescol_sb[:],
                                 rhs=kf_sb[:, 4 * g:4 * (g + 1)],
                                 start=True, stop=True)
                nc.vector.tensor_copy(tks_sb[:, 4 * g:4 * (g + 1)], tkp[:])

            # per-tile exclusive prefix of kept counts: shift-add scan on [1,16]
            posb_sb = sb.tile([1, NTILE], F32, tag="posb")
            nc.vector.memset(posb_sb[:, :1], 0.0)
            nc.vector.tensor_copy(posb_sb[:, 1:], tks_sb[:, :NTILE - 1])
            cur = posb_sb
            for sh in (1, 2, 4, 8):
                nxt = sb.tile([1, NTILE], F32, tag=f"posb{sh}")
                nc.vector.tensor_copy(nxt[:, :sh], cur[:, :sh])
                nc.vector.tensor_tensor(nxt[:, sh:], cur[:, sh:],
                                        cur[:, :NTILE - sh], op=ALU.add)
                cur = nxt
            posb_sb = cur
            nc.vector.tensor_scalar_add(posb_sb[:], posb_sb[:], -1.0)

            # ============== PHASE B3: compaction matmuls ================
            # cmpT[j, r] accumulates [token-idx ; gate-scale] for the r-th
            # kept token; lhsT is the tiny [128,2] value pair so the big
            # one-hot M matrix streams as the moving operand.
            tsv_sb = sb.tile([P, 2 * NTILE], F32, tag="tsv")
            tsv3 = tsv_sb[:].rearrange("p (i j) -> p i j", j=2)
            nc.vector.tensor_copy(
                tsv3[:, :, 0:1],
                tidx_sb[:].rearrange("p (i o) -> p i o", o=1))
            nc.vector.tensor_copy(
                tsv3[:, :, 1:2],
                s_sb[:].rearrange("p (i o) -> p i o", o=1))
            cmpT = pcmp.tile([2, KMAX], F32, space="PSUM", tag="cmpT")
            for g in range(4):
                pos4 = psml.tile([P, 4], F32, space="PSUM", tag="sm")
                nc.tensor.matmul(pos4[:], lhsT=tri_sb[:],
                                 rhs=kf_sb[:, 4 * g:4 * (g + 1)],
                                 start=True, stop=False)
                nc.tensor.matmul(pos4[:], lhsT=ones1_sb[:],
                                 rhs=posb_sb[0:1, 4 * g:4 * (g + 1)],
                                 start=False, stop=True)
                notk4 = st.tile([P, 4], F32, tag="notk4")
                nc.vector.tensor_scalar(notk4[:], kf_sb[:, 4 * g:4 * (g + 1)],
                                        0.5, None, op0=ALU.is_lt)
                nc.vector.tensor_scalar_mul(notk4[:], notk4[:], BIG)
                poss4 = st.tile([P, 4], F32, tag="poss4")
                nc.vector.tensor_tensor(poss4[:], pos4[:], notk4[:], op=ALU.add)
                for j in range(4):
                    i = 4 * g + j
                    M = st.tile([P, KMAX], F32, tag="M")
                    nc.vector.tensor_scalar(M[:], iota_sb[:], poss4[:, j:j + 1],
                                            None, op0=ALU.is_equal)
                    nc.tensor.matmul(cmpT[:], lhsT=tsv_sb[:, 2 * i:2 * i + 2],
                                     rhs=M[:],
                                     start=(i == 0), stop=(i == NTILE - 1))

            # extract compaction results: transpose [2, 128] -> [128, 2]
            # on the PE instead of a 3-DMA DRAM bounce
            cmpT_sb = sb.tile([2, KMAX], F32, tag="cmpTsb")
            nc.vector.tensor_copy(cmpT_sb[:], cmpT[:])
            gst = psml.tile([P, 2], F32, space="PSUM", tag="sm")
            nc.tensor.transpose(gst[:], cmpT_sb[:], ident_sb[:2, :2])
            gs_sb = sb.tile([P, 2], F32, tag="gs")   # col 0 = idx, 1 = s
            nc.vector.tensor_copy(gs_sb[:], gst[:])
            halves = []
            for half in range(1):
                scmp = gs_sb[:, 1:2]
                gidx = sb.tile([P, 1], I32, tag=f"gidx{half}")
                nc.vector.tensor_copy(gidx[:], gs_sb[:, 0:1])
                padf = st.tile([P, 1], F32, tag="padf")
                nc.vector.tensor_scalar(padf[:], scmp, 0.0, None,
                                        op0=ALU.is_equal)
                nc.vector.tensor_scalar_mul(padf[:], padf[:], BIG)
                gsf = st.tile([P, 1], F32, tag="gsf")
                nc.vector.tensor_tensor(gsf[:], gs_sb[:, 0:1],
                                        padf[:], op=ALU.add)
                sidx = sb.tile([P, 1], I32, tag=f"sidx{half}")
                nc.vector.tensor_copy(sidx[:], gsf[:])
                halves.append((scmp, gidx, sidx))

        # ============== PHASE C: gather, expert matmul, scatter =========
        with ExitStack() as pc:
            pbig = pc.enter_context(tc.tile_pool(name="pbig2", bufs=2,
                                                 space="PSUM"))
            pout = pc.enter_context(tc.tile_pool(name="pout", bufs=2,
                                                 space="PSUM"))
            for half, (scmp, gidx, sidx) in enumerate(halves):
                xg = st.tile([P, H], F32, tag="xg")
                nc.gpsimd.indirect_dma_start(
                    out=xg[:], out_offset=None, in_=x[:, :],
                    in_offset=bass.IndirectOffsetOnAxis(ap=gidx[:, :1],
                                                        axis=0))
                nc.vector.tensor_scalar_mul(xg[:], xg[:], scmp[:, :1])
                xgT = st.tile([P, H], EDT, tag="xgT")
                for g2 in range(2):
                    tp = pbig.tile([P, 512], F32, space="PSUM", tag="tp2")
                    for c4 in range(4):
                        c = g2 * 4 + c4
                        nc.tensor.transpose(tp[:, c4 * P:(c4 + 1) * P],
                                            xg[:, c * P:(c + 1) * P],
                                            ident_sb[:])
                    nc.vector.tensor_copy(xgT[:, g2 * 512:(g2 + 1) * 512],
                                          tp[:])
                stp = pout.tile([1, P], F32, space="PSUM", tag="stp")
                nc.tensor.transpose(stp[:], scmp[:, :1], ident_sb[:])
                sT = sb.tile([1, P], EDT, tag=f"sT{half}")
                nc.vector.tensor_copy(sT[:], stp[:])

                outsb = st.tile([P, H], F32, tag="outsb")
                for n in range(2):
                    po = pout.tile([P, 512], F32, space="PSUM", tag="po")
                    for c in range(NCH):
                        nc.tensor.matmul(
                            po[:], lhsT=xgT[:, c * P:(c + 1) * P],
                            rhs=we_sb[:, c * H + n * 512: c * H + (n + 1) * 512],
                            start=(c == 0), stop=False)
                    nc.tensor.matmul(po[:], lhsT=sT[:],
                                     rhs=be_sb[0:1, n * 512:(n + 1) * 512],
                                     start=False, stop=True)
                    nc.vector.tensor_copy(outsb[:, n * 512:(n + 1) * 512],
                                          po[:])
                nc.gpsimd.indirect_dma_start(
                    out=out[:, :],
                    out_offset=bass.IndirectOffsetOnAxis(ap=sidx[:, :1],
                                                         axis=0),
                    in_=outsb[:], in_offset=None,
                    bounds_check=T_LOC - 1, oob_is_err=False)


# ---------------------------------------------------------------------------
# host side
# ---------------------------------------------------------------------------

def make_consts():
    tri = np.triu(np.ones((P, P), np.float32))            # tri[tp,t]=1 if tp<=t
    ident = np.eye(P, dtype=np.float32)
    iota = np.tile(np.arange(KMAX, dtype=np.float32)[None, :], (P, 1))
    tidx = (np.arange(NTILE, dtype=np.float32)[None, :] * P
            + np.arange(P, dtype=np.float32)[:, None])
    blk_of = np.arange(P) // BLK                          # token row -> block-in-tile
    esum = (blk_of[:, None] == np.arange(4)[None, :]).astype(np.float32)
    eexp = esum.T.copy()
    ones1 = np.ones((1, P), np.float32)
    onescol = np.ones((P, 1), np.float32)
    neg1 = -np.ones((1, E), np.float32)
    j = np.arange(NBLK)
    h64 = (j[:, None] < 4 * (j[None, :] // 4)).astype(np.float32)
    i16 = np.arange(NTILE)
    tri16 = (i16[:, None] < i16[None, :]).astype(np.float32)
    return dict(tri128=tri, ident=ident, iota256=iota, tidx16=tidx,
                esum=esum, eexp=eexp, ones1=ones1, onescol=onescol,
                neg1=neg1, h64=h64, tri16=tri16)


def make_maskk(k):
    # rows (r*64 + jp) = foreign core r's local block jp (global block 8*jp+r)
    # cols j = my local block (global 8*j + k)
    r = np.arange(NCORES)[:, None, None]
    jp = np.arange(NBLK)[None, :, None]
    jm = np.arange(NBLK)[None, None, :]
    m = (r != k) & (8 * jp + r < 8 * jm + k)
    return m.astype(np.float32).reshape(NCORES * NBLK, NBLK)


def make_in_maps(x, w_gate, w_expert, b_expert):
    xf = np.ascontiguousarray(np.asarray(x, np.float32).reshape(-1, H))
    xb = xf.reshape(-1, BLK, H)          # (512, 32, H)
    consts = make_consts()
    wgf = np.ascontiguousarray(np.asarray(w_gate, np.float32))
    wef = np.ascontiguousarray(np.asarray(w_expert, np.float32))
    bef = np.ascontiguousarray(np.asarray(b_expert, np.float32).reshape(1, H))
    in_maps = []
    for k in range(NCORES):
        shard = np.ascontiguousarray(xb[k::NCORES].reshape(T_LOC, H))
        m = {"x": shard, "xt": np.ascontiguousarray(shard.T),
             "w_gate": wgf, "w_expert": wef, "b_expert": bef,
             "maskk": make_maskk(k)}
        m.update(consts)
        in_maps.append(m)
    return in_maps


def assemble_out(results, batch_shape):
    T = NCORES * T_LOC
    outf = np.empty((T // BLK, BLK, H), np.float32)
    for k in range(NCORES):
        outf[k::NCORES] = results[k]["out"].reshape(-1, BLK, H)
    return outf.reshape(batch_shape)


_NC = None
LAST_EXEC_NS = None


def _maybe_register_ntff_hook():
    """Best-effort registration of the axon NTFF profiling hook (used only
    when BASS_TRACE is set); harmless if unavailable."""
    try:
        import antenv
        from trn_agent_boot.trn_boot import _ntff_profile_via_ctypes
        if "antenv.axon_hooks" in sys.modules:
            return
        hook = _ntff_profile_via_ctypes("/opt/axon/libaxon_pjrt.so")
        mod = types.ModuleType("antenv.axon_hooks")
        mod.get_axon_ntff_profile_hook = lambda: hook
        mod.set_axon_ntff_profile_hook = lambda h: None
        antenv.axon_hooks = mod
        sys.modules["antenv.axon_hooks"] = mod
        bass_utils.upload_artifacts = lambda tmpdir: f"file://{tmpdir}"
    except Exception:
        pass


def kernel(x, w_gate, w_expert, b_expert):
    global _NC, LAST_EXEC_NS
    if os.environ.get("BASS_TRACE"):
        _maybe_register_ntff_hook()
    if _NC is None:
        _NC = build()
    in_maps = make_in_maps(x, w_gate, w_expert, b_expert)
    # The fleet occasionally throws a transient NRT_EXEC_UNIT_UNRECOVERABLE
    # on execute (observed ~10% of invocations; always recovers on retry).
    last_exc = None
    for attempt in range(3):
        try:
            res = bass_utils.run_bass_kernel_spmd(
                _NC, in_maps, core_ids=list(range(NCORES)))
            break
        except Exception as exc:
            last_exc = exc
            import time as _time
            _time.sleep(2.0)
    else:
        raise last_exc
    LAST_EXEC_NS = res.exec_time_ns
    return assemble_out(res.results, np.asarray(x).shape)



# revision 10
# speedup vs baseline: 1.0013x; 1.0013x over previous
"""Distributed sparse-MoE routing kernel for 8 Trainium2 NeuronCores (v2).

Algorithm notes
---------------
The reference routes T=16384 tokens (top-1 of E=8 experts, capacity C=100,
tokens past capacity dropped) and applies ONE shared expert weight (H -> H
Linear).  Because the expert weight is shared, the output collapses to

    out[t] = gate_t * (x_t @ W + b)   if token t wins a capacity slot
           = 0                        otherwise

so only <= E*C = 800 of 16384 tokens need the big matmul.  Token t (choosing
expert e) wins a slot iff fewer than C earlier tokens (global order) chose e.

Distribution: tokens sharded over 8 cores in 32-token blocks, strided (core k
owns blocks b with b % 8 == k).  Each core computes per-block expert counts
locally; one small AllGather (64x8 per core) shares them; small matmuls turn
the gathered table into per-block capacity bases.  Everything else is local.

v2 changes vs the 125-138us v1 (trace-driven):
- Router runs as an fp16 hi/lo split instead of fp32: x = x_h + x_l and
  w_gate = w_h + w_l (fp16 pairs, exact decomposition).  Stationary is
  [w_h | w_l] packed [128,16]; streaming x_h then x_l accumulates all four
  cross terms in one PSUM, one vector add folds the halves.  fp16 streams at
  1 cycle/row vs fp32's 4, so the router drops ~30us -> ~8us of PE time.
  Measured offline on the seed-0 data: max logit error 2.9e-6 vs min top-2
  gap 1.39e-5 (16x margin), zero argmax flips.
- Softmax/masks batched over all 16 token tiles in one pass (was 4 groups).
- Post-AG compaction batched: one-shot keep/kf/s, one [128,16] pos matmul
  pair, 16 back-to-back M-builds then 16 back-to-back cmpT matmuls (was an
  8us vector<->PE ping-pong).
- Gate scale folded into the expert-output PSUM->SBUF copy (tensor_scalar
  mult), so the gathered x rows go straight to PE transposes.
- Scatter pads route to a dummy DRAM row (out has 2049 rows; pad slots
  accumulate idx 0 and add 2048) so no bounds-check path is needed.
- Constants consolidated into two blob DMAs (hot: ident+esum needed during
  the router; cold: everything post-AG) instead of ~12 tiny DMAs at ~0.6us
  of sync-engine trigger time each.

Measured constraints on this fleet (do not re-derive):
- A collective costs ~60-85us in an empty kernel; in v1 the AG trigger at
  ~54us completed at ~82us (15.8us peer/launch skew + 12us mesh).
- The PE duty-cycles to 50% (HAM k=4/n=8) after ~30us of sustained fp32
  work; lighter fp16 router reduces this pressure.
- DMA: 16 HW queues, ~22GB/s each when all contend (358GB/s/core cap);
  each dma_start costs ~0.61us of sync-engine trigger time, so batch
  small transfers but keep big ones spread across queues.
- exec_time is core 0's span (only core 0 is profiled by default).
"""
import os
import sys
import types
from contextlib import ExitStack

sys.path.insert(0, "/opt/trn_rl_repo")

import numpy as np

import concourse.bass as bass
import concourse.bacc as bacc
import concourse.mybir as mybir
import concourse.tile as tile
from concourse import bass_utils

F32 = mybir.dt.float32
F32R = mybir.dt.float32r
F16 = mybir.dt.float16
I32 = mybir.dt.int32
AX = mybir.AxisListType
ALU = mybir.AluOpType
ACT = mybir.ActivationFunctionType

P = 128          # SBUF partitions / tile rows
H = 1024         # hidden dim
E = 8            # experts
C = 100          # capacity
NCORES = 8
T_LOC = 2048     # tokens per core
NTILE = T_LOC // P   # 16 token tiles per core
NCH = H // P         # 8 hidden chunks
BLK = 32             # token block size for sharding
NBLK = T_LOC // BLK  # 64 local blocks per core
KMAX = 128           # max compacted (kept) tokens per core (<=114 actual)
TRASH = T_LOC        # dummy out row for pad slots

# cold consts blob column offsets
CC_TRI = 0
CC_IOTA = 128
CC_TIDX = 256
CC_MISC = 272        # rows 0-3 eexp
CC_ONESC = 400
CC_NEG1 = 401
CC_H64 = 409
CC_MASKK = 473
CC_ONES1 = 473 + 4 * NBLK    # 729, row 0 all-ones [1, 128]
CC_W = CC_ONES1 + 128        # 857


def build(has_bias=False):
    """Build + compile the SPMD program (identical on all 8 cores)."""
    nc = bacc.Bacc("TRN2", target_bir_lowering=False, debug=False,
                   num_devices=NCORES)

    x = nc.dram_tensor("x", [T_LOC, H], F32, kind="ExternalInput")
    xth = nc.dram_tensor("xth", [H, T_LOC], F16, kind="ExternalInput")
    xtl = nc.dram_tensor("xtl", [H, T_LOC], F16, kind="ExternalInput")
    wgcat = nc.dram_tensor("wgcat", [H, 2 * E], F16, kind="ExternalInput")
    we = nc.dram_tensor("w_expert", [H, H], F32R, kind="ExternalInput")
    be = (nc.dram_tensor("b_expert", [1, H], F32R, kind="ExternalInput")
          if has_bias else None)
    ch = nc.dram_tensor("ch", [P, 132], F32, kind="ExternalInput")
    cc = nc.dram_tensor("cc", [P, CC_W], F32, kind="ExternalInput")

    out = nc.dram_tensor("out", [T_LOC + 1, H], F32, kind="ExternalOutput")

    with tile.TileContext(nc) as tc:
        _body(nc, tc, x, xth, xtl, wgcat, we, be, ch, cc, out)

    nc.compile()
    return nc


def _body(nc, tc, x, xth, xtl, wgcat, we, be, ch, cc, out):
    with ExitStack() as top:
        sb = top.enter_context(tc.tile_pool(name="sb", bufs=1))
        st = top.enter_context(tc.tile_pool(name="st", bufs=4))
        dram = top.enter_context(tc.tile_pool(name="dram", bufs=1, space="DRAM"))

        # ---- DMA schedule -------------------------------------------------
        # Trigger order is program order on the sync engine (~0.61us each).
        # wgcat + hot consts first (router + transposes need them), then the
        # x halves interleaved by group so the router starts on group 0
        # early, then cold consts (first used in the AG stall), then w_expert
        # (first used in phase C).
        wg_sb = sb.tile([P, NCH * 2 * E], F16, tag="wgcat")
        nc.sync.dma_start(wg_sb[:].rearrange("p (c e) -> p c e", c=NCH),
                          wgcat[:, :].rearrange("(c p) e -> p c e", p=P))
        ch_sb = sb.tile([P, 132], F32, tag="ch")
        nc.sync.dma_start(ch_sb[:], ch[:, :])

        xh_sb = sb.tile([P, NCH * T_LOC], F16, tag="xh")
        xl_sb = sb.tile([P, NCH * T_LOC], F16, tag="xl")
        xh3 = xh_sb[:].rearrange("p (c t) -> p c t", c=NCH)
        xl3 = xl_sb[:].rearrange("p (c t) -> p c t", c=NCH)
        cold_done = False
        cc_sb = sb.tile([P, CC_W], F32, tag="cc")
        for g in range(4):
            for src, dst in ((xth, xh3), (xtl, xl3)):
                for q in range(2):
                    nc.sync.dma_start(
                        dst[:, 4 * q:4 * q + 4, g * 512:(g + 1) * 512],
                        src[4 * q * P:(4 * q + 4) * P,
                            g * 512:(g + 1) * 512].rearrange(
                                "(c p) t -> p c t", p=P))
            if g == 1 and not cold_done:
                nc.sync.dma_start(cc_sb[:], cc[:, :])
                cold_done = True

        we_sb = sb.tile([P, NCH * H], F32R, tag="we")
        for c in range(NCH):
            nc.sync.dma_start(we_sb[:, c * H:(c + 1) * H],
                              we[c * P:(c + 1) * P, :])
        be_sb = None
        if be is not None:
            be_sb = sb.tile([1, H], F32R, tag="be")
            nc.sync.dma_start(be_sb[:], be[:, :])

        # const views
        ident = ch_sb[:, 0:128]
        esum = ch_sb[:, 128:132]
        tri = cc_sb[:, CC_TRI:CC_TRI + 128]
        iota = cc_sb[:, CC_IOTA:CC_IOTA + 128]
        tidx = cc_sb[:, CC_TIDX:CC_TIDX + NTILE]
        eexp = cc_sb[0:4, CC_MISC:CC_MISC + 128]
        ones1 = cc_sb[0:1, CC_ONES1:CC_ONES1 + 128]
        onescol = cc_sb[:, CC_ONESC:CC_ONESC + 1]
        neg1 = cc_sb[0:1, CC_NEG1:CC_NEG1 + E]

        # ---- persistent per-token state ----------------------------------
        masks_sb = sb.tile([P, NTILE * E], F32, tag="masks")
        gate_sb = sb.tile([P, NTILE], F32, tag="gate")
        s_sb = sb.tile([P, NTILE], F32, tag="s")
        kf_sb = sb.tile([P, NTILE], F32, tag="kf")
        bc_sb = sb.tile([4, NTILE * E], F32, tag="bc")

        # ================= PHASE A: router + masks + counts ===============
        logits_sb = sb.tile([P, NTILE * E], F32, tag="logits")
        big_sb = sb.tile([P, NTILE * 2 * E], F32, tag="big")
        with ExitStack() as pa:
            plg = pa.enter_context(tc.tile_pool(name="plg", bufs=1, space="PSUM"))
            ptp = pa.enter_context(tc.tile_pool(name="ptp", bufs=2, space="PSUM"))
            psml = pa.enter_context(tc.tile_pool(name="psml", bufs=1, space="PSUM"))

            # router: 4 groups x 16 fp16 matmuls, all accumulated per group
            lgss = []
            for g in range(4):
                lgT = plg.tile([2 * E, 512], F32, space="PSUM", tag=f"lgT{g}")
                for c in range(NCH):
                    nc.tensor.matmul(
                        lgT[:], lhsT=wg_sb[:, c * 2 * E:(c + 1) * 2 * E],
                        rhs=xh3[:, c, g * 512:(g + 1) * 512],
                        start=(c == 0), stop=False)
                    nc.tensor.matmul(
                        lgT[:], lhsT=wg_sb[:, c * 2 * E:(c + 1) * 2 * E],
                        rhs=xl3[:, c, g * 512:(g + 1) * 512],
                        start=False, stop=(c == NCH - 1))
                lgs = st.tile([2 * E, 512], F32, tag="lgs")
                nc.vector.tensor_copy(lgs[:], lgT[:])
                lgss.append(lgs)

            # transpose all 16 tiles back: [16, 128] -> [128, 16], batched 4
            for b in range(4):
                tp4 = ptp.tile([P, 4 * 2 * E], F32, space="PSUM", tag="tp4")
                for j in range(4):
                    i = 4 * b + j
                    nc.tensor.transpose(
                        tp4[:, j * 2 * E:(j + 1) * 2 * E],
                        lgss[i // 4][:, (i % 4) * P:(i % 4 + 1) * P],
                        ident[:2 * E, :2 * E])
                nc.vector.tensor_copy(
                    big_sb[:, b * 4 * 2 * E:(b + 1) * 4 * 2 * E], tp4[:])

            # fold hi/lo halves: logits[:, (i,e)] = big[:, (i, e)] + big[:, (i, 8+e)]
            big3 = big_sb[:].rearrange("p (i e) -> p i e", e=2 * E)
            lg3 = logits_sb[:].rearrange("p (i e) -> p i e", e=E)
            nc.vector.tensor_tensor(lg3, big3[:, :, 0:E], big3[:, :, E:2 * E],
                                    op=ALU.add)

            # batched softmax / first-max mask over [128, 16, 8]
            m16 = st.tile([P, NTILE], F32, tag="m16")
            nc.vector.reduce_max(m16[:], lg3, axis=AX.X)
            m16b = m16[:].rearrange("p (i o) -> p i o", o=1).to_broadcast(
                [P, NTILE, E])
            d128 = st.tile([P, NTILE * E], F32, tag="d128")
            nc.vector.tensor_tensor(
                d128[:].rearrange("p (i e) -> p i e", e=E), lg3, m16b,
                op=ALU.subtract)
            e128 = st.tile([P, NTILE * E], F32, tag="e128")
            nc.scalar.activation(e128[:], d128[:], ACT.Exp)
            z16 = st.tile([P, NTILE], F32, tag="z16")
            nc.vector.reduce_sum(
                z16[:], e128[:].rearrange("p (i e) -> p i e", e=E), axis=AX.X)
            nc.vector.reciprocal(gate_sb[:], z16[:])
            mraw = st.tile([P, NTILE * E], F32, tag="mraw")
            nc.vector.tensor_tensor(
                mraw[:].rearrange("p (i e) -> p i e", e=E), lg3, m16b,
                op=ALU.is_equal)
            c1 = mraw
            for sh in (1, 2, 4):
                c2 = st.tile([P, NTILE * E], F32, tag=f"cc{sh}")
                c1v = c1[:].rearrange("p (i e) -> p i e", e=E)
                c2v = c2[:].rearrange("p (i e) -> p i e", e=E)
                nc.vector.tensor_copy(c2v[:, :, :sh], c1v[:, :, :sh])
                nc.vector.tensor_tensor(c2v[:, :, sh:], c1v[:, :, sh:],
                                        c1v[:, :, :E - sh], op=ALU.add)
                c1 = c2
            nc.vector.tensor_scalar(masks_sb[:], c1[:], 1.0, None,
                                    op0=ALU.is_equal)
            nc.vector.tensor_tensor(masks_sb[:], masks_sb[:], mraw[:],
                                    op=ALU.mult)

            # per-block expert counts, one matmul
            bcp = psml.tile([4, NTILE * E], F32, space="PSUM", tag="bcp")
            nc.tensor.matmul(bcp[:], lhsT=esum, rhs=masks_sb[:],
                             start=True, stop=True)
            nc.vector.tensor_copy(bc_sb[:], bcp[:])

        # ================= AllGather of per-block counts ==================
        agin = dram.tile([NBLK, E], F32, tag="agin")
        agout = dram.tile([NCORES * NBLK, E], F32, tag="agout")
        scr = dram.tile([NBLK, E], F32, tag="scr")
        nc.sync.dma_start(agin[:].rearrange("(i q) e -> q i e", q=4),
                          bc_sb[:].rearrange("p (i e) -> p i e", e=E))
        nc.gpsimd.collective_compute(
            "AllGather", ALU.bypass,
            replica_groups=[list(range(NCORES))],
            ins=[agin[:].opt()], outs=[agout[:].opt()])
        agt_sb = sb.tile([P, 4 * E], F32, tag="agt")
        nc.sync.dma_start(agt_sb[:].rearrange("p (c e) -> p c e", c=4),
                          agout[:].rearrange("(c p) e -> p c e", p=P))
        bc64_sb = sb.tile([NBLK, E], F32, tag="bc64")
        nc.sync.dma_start(bc64_sb[:], agin[:])

        with ExitStack() as pb:
            psml = pb.enter_context(tc.tile_pool(name="psml2", bufs=2, space="PSUM"))
            ploc = pb.enter_context(tc.tile_pool(name="ploc", bufs=1, space="PSUM"))
            pcmp = pb.enter_context(tc.tile_pool(name="pcmp", bufs=1, space="PSUM"))

            # AG-independent matmuls first: they run inside the stall.
            loc4s = []
            for g in range(4):
                loc4 = ploc.tile([P, 32], F32, space="PSUM", tag=f"loc{g}")
                nc.tensor.matmul(loc4[:], lhsT=tri,
                                 rhs=masks_sb[:, 32 * g:32 * (g + 1)],
                                 start=True, stop=False)
                loc4s.append(loc4)
            ab = psml.tile([NBLK, E], F32, space="PSUM", tag="sm")
            nc.tensor.matmul(ab[:], lhsT=cc_sb[0:64, CC_H64:CC_H64 + NBLK],
                             rhs=bc64_sb[:], start=True, stop=False)
            nc.tensor.matmul(ab[:], lhsT=ones1[:, :NBLK], rhs=neg1,
                             start=False, stop=False)
            for c in range(4):
                nc.tensor.matmul(
                    ab[:],
                    lhsT=cc_sb[:, CC_MASKK + c * NBLK:CC_MASKK + (c + 1) * NBLK],
                    rhs=agt_sb[:, c * E:(c + 1) * E],
                    start=False, stop=(c == 3))
            ab_sb = sb.tile([NBLK, E], F32, tag="ab64")
            nc.vector.tensor_copy(ab_sb[:], ab[:])
            nc.sync.dma_start(scr[:], ab_sb[:])
            addbase_sb = sb.tile([4, NTILE * E], F32, tag="addbase")
            nc.sync.dma_start(addbase_sb[:].rearrange("p (i e) -> p i e", e=E),
                              scr[:].rearrange("(i q) e -> q i e", q=4))

            # ============== PHASE B: keep / gate-scale / compaction =======
            keep_sb = sb.tile([P, NTILE * E], F32, tag="keep")
            for g in range(4):
                loc4 = loc4s[g]
                nc.tensor.matmul(loc4[:], lhsT=eexp,
                                 rhs=addbase_sb[:, 32 * g:32 * (g + 1)],
                                 start=False, stop=True)
                nc.vector.tensor_scalar(keep_sb[:, 32 * g:32 * (g + 1)],
                                        loc4[:], float(C), None, op0=ALU.is_lt)
            nc.vector.tensor_tensor(keep_sb[:], keep_sb[:], masks_sb[:],
                                    op=ALU.mult)
            keep3 = keep_sb[:].rearrange("p (i e) -> p i e", e=E)
            nc.vector.reduce_sum(kf_sb[:], keep3, axis=AX.X)
            g16b = gate_sb[:].rearrange("p (i o) -> p i o", o=1).to_broadcast(
                [P, NTILE, E])
            s128 = st.tile([P, NTILE * E], F32, tag="s128")
            nc.vector.tensor_tensor(
                s128[:].rearrange("p (i e) -> p i e", e=E), keep3, g16b,
                op=ALU.mult)
            nc.vector.reduce_sum(
                s_sb[:], s128[:].rearrange("p (i e) -> p i e", e=E), axis=AX.X)
            tkp = psml.tile([1, NTILE], F32, space="PSUM", tag="sm")
            nc.tensor.matmul(tkp[:], lhsT=onescol, rhs=kf_sb[:],
                             start=True, stop=True)
            tks_sb = sb.tile([1, NTILE], F32, tag="tks")
            nc.vector.tensor_copy(tks_sb[:], tkp[:])

            # exclusive prefix over 16 tile-counts
            posb_sb = sb.tile([1, NTILE], F32, tag="posb")
            nc.vector.memset(posb_sb[:, :1], 0.0)
            nc.vector.tensor_copy(posb_sb[:, 1:], tks_sb[:, :NTILE - 1])
            cur = posb_sb
            for sh in (1, 2, 4, 8):
                nxt = sb.tile([1, NTILE], F32, tag=f"posb{sh}")
                nc.vector.tensor_copy(nxt[:, :sh], cur[:, :sh])
                nc.vector.tensor_tensor(nxt[:, sh:], cur[:, sh:],
                                        cur[:, :NTILE - sh], op=ALU.add)
                cur = nxt
            posb_sb = cur
            nc.vector.tensor_scalar_add(posb_sb[:], posb_sb[:], -1.0)

            # slot position per token: within-tile rank + tile base
            pos16 = psml.tile([P, NTILE], F32, space="PSUM", tag="sm")
            nc.tensor.matmul(pos16[:], lhsT=tri, rhs=kf_sb[:],
                             start=True, stop=False)
            nc.tensor.matmul(pos16[:], lhsT=ones1, rhs=posb_sb[0:1, :],
                             start=False, stop=True)
            notk = st.tile([P, NTILE], F32, tag="notk")
            nc.vector.tensor_scalar(notk[:], kf_sb[:], 0.5, None,
                                    op0=ALU.is_lt)
            nc.vector.tensor_scalar_mul(notk[:], notk[:], 4096.0)
            poss = st.tile([P, NTILE], F32, tag="poss")
            nc.vector.tensor_tensor(poss[:], pos16[:], notk[:], op=ALU.add)

            # value pairs [token-idx ; gate-scale] per tile
            tsv_sb = sb.tile([P, 2 * NTILE], F32, tag="tsv")
            tsv3 = tsv_sb[:].rearrange("p (i j) -> p i j", j=2)
            nc.vector.tensor_copy(
                tsv3[:, :, 0:1], tidx.rearrange("p (i o) -> p i o", o=1))
            nc.vector.tensor_copy(
                tsv3[:, :, 1:2], s_sb[:].rearrange("p (i o) -> p i o", o=1))

            # one-hot M for all tiles (16 back-to-back vector ops), then the
            # 16 compaction matmuls back-to-back
            M_all = sb.tile([P, NTILE * KMAX], F32, tag="Mall")
            for i in range(NTILE):
                nc.vector.tensor_scalar(M_all[:, i * KMAX:(i + 1) * KMAX],
                                        iota, poss[:, i:i + 1], None,
                                        op0=ALU.is_equal)
            cmpT = pcmp.tile([2, KMAX], F32, space="PSUM", tag="cmpT")
            for i in range(NTILE):
                nc.tensor.matmul(cmpT[:], lhsT=tsv_sb[:, 2 * i:2 * i + 2],
                                 rhs=M_all[:, i * KMAX:(i + 1) * KMAX],
                                 start=(i == 0), stop=(i == NTILE - 1))

            # extract: transpose [2,128] -> [128,2] on the PE
            cmpT_sb = sb.tile([2, KMAX], F32, tag="cmpTsb")
            nc.vector.tensor_copy(cmpT_sb[:], cmpT[:])
            gst = psml.tile([P, 2], F32, space="PSUM", tag="sm")
            nc.tensor.transpose(gst[:], cmpT_sb[:], ident[:2, :2])
            gs_sb = sb.tile([P, 2], F32, tag="gs")   # col 0 = idx, 1 = s
            nc.vector.tensor_copy(gs_sb[:], gst[:])
            scmp = gs_sb[:, 1:2]
            gidx = sb.tile([P, 1], I32, tag="gidx")
            nc.vector.tensor_copy(gidx[:], gs_sb[:, 0:1])
            # pad slots have idx 0 and s 0 -> route them to the trash row
            padf = st.tile([P, 1], F32, tag="padf")
            nc.vector.tensor_scalar(padf[:], scmp, 0.0, None,
                                    op0=ALU.is_equal)
            nc.vector.tensor_scalar_mul(padf[:], padf[:], float(TRASH))
            gsf = st.tile([P, 1], F32, tag="gsf")
            nc.vector.tensor_tensor(gsf[:], gs_sb[:, 0:1], padf[:],
                                    op=ALU.add)
            sidx = sb.tile([P, 1], I32, tag="sidx")
            nc.vector.tensor_copy(sidx[:], gsf[:])

        # ============== PHASE C: gather, expert matmul, scatter ===========
        with ExitStack() as pc:
            ptp = pc.enter_context(tc.tile_pool(name="ptp2", bufs=2,
                                                space="PSUM"))
            ppo = pc.enter_context(tc.tile_pool(name="ppo", bufs=2,
                                                space="PSUM"))
            xg = st.tile([P, H], F32, tag="xg")
            nc.gpsimd.indirect_dma_start(
                out=xg[:], out_offset=None, in_=x[:, :],
                in_offset=bass.IndirectOffsetOnAxis(ap=gidx[:, :1], axis=0))
            xgT = st.tile([P, H], F32R, tag="xgT")
            for g2 in range(2):
                tp = ptp.tile([P, 512], F32, space="PSUM", tag="tp2")
                for c4 in range(4):
                    c = g2 * 4 + c4
                    nc.tensor.transpose(tp[:, c4 * P:(c4 + 1) * P],
                                        xg[:, c * P:(c + 1) * P], ident)
                nc.vector.tensor_copy(xgT[:, g2 * 512:(g2 + 1) * 512], tp[:])

            outsb = st.tile([P, H], F32, tag="outsb")
            for n in range(2):
                po = ppo.tile([P, 512], F32, space="PSUM", tag="po")
                for c in range(NCH):
                    nc.tensor.matmul(
                        po[:], lhsT=xgT[:, c * P:(c + 1) * P],
                        rhs=we_sb[:, c * H + n * 512: c * H + (n + 1) * 512],
                        start=(c == 0), stop=(be_sb is None and c == NCH - 1))
                if be_sb is not None:
                    nc.tensor.matmul(po[:], lhsT=ones1.bitcast(F32R),
                                     rhs=be_sb[0:1, n * 512:(n + 1) * 512],
                                     start=False, stop=True)
                # gate scale folded into the PSUM->SBUF copy
                nc.vector.tensor_scalar(outsb[:, n * 512:(n + 1) * 512],
                                        po[:], scmp[:, :1], None,
                                        op0=ALU.mult)
            nc.gpsimd.indirect_dma_start(
                out=out[:, :],
                out_offset=bass.IndirectOffsetOnAxis(ap=sidx[:, :1], axis=0),
                in_=outsb[:], in_offset=None)


# ---------------------------------------------------------------------------
# host side
# ---------------------------------------------------------------------------

def make_consts():
    ident = np.eye(P, dtype=np.float32)
    blk_of = np.arange(P) // BLK
    esum = (blk_of[:, None] == np.arange(4)[None, :]).astype(np.float32)
    ch = np.concatenate([ident, esum], axis=1)           # [128, 132]

    cold = np.zeros((P, CC_W), np.float32)
    cold[:, CC_TRI:CC_TRI + 128] = np.triu(np.ones((P, P), np.float32))
    cold[:, CC_IOTA:CC_IOTA + 128] = np.tile(
        np.arange(KMAX, dtype=np.float32)[None, :], (P, 1))
    cold[:, CC_TIDX:CC_TIDX + NTILE] = (
        np.arange(NTILE, dtype=np.float32)[None, :] * P
        + np.arange(P, dtype=np.float32)[:, None])
    cold[0:4, CC_MISC:CC_MISC + 128] = esum.T
    cold[0:1, CC_ONES1:CC_ONES1 + 128] = 1.0
    cold[:, CC_ONESC] = 1.0
    cold[0:1, CC_NEG1:CC_NEG1 + E] = -1.0
    j = np.arange(NBLK)
    cold[0:64, CC_H64:CC_H64 + NBLK] = (
        j[:, None] < 4 * (j[None, :] // 4)).astype(np.float32)
    return ch, cold


def make_maskk(k):
    # rows (r*64 + jp) = foreign core r's local block jp (global block 8*jp+r)
    # cols j = my local block (global 8*j + k)
    r = np.arange(NCORES)[:, None, None]
    jp = np.arange(NBLK)[None, :, None]
    jm = np.arange(NBLK)[None, None, :]
    m = (r != k) & (8 * jp + r < 8 * jm + k)
    return m.astype(np.float32).reshape(NCORES * NBLK, NBLK)


def make_in_maps(x, w_gate, w_expert, b_expert):
    xf = np.ascontiguousarray(np.asarray(x, np.float32).reshape(-1, H))
    xb = xf.reshape(-1, BLK, H)          # (512, 32, H)
    ch, cold = make_consts()
    wgf = np.asarray(w_gate, np.float32)
    wg_h = wgf.astype(np.float16)
    wg_l = (wgf - wg_h.astype(np.float32)).astype(np.float16)
    wgcat = np.ascontiguousarray(np.concatenate([wg_h, wg_l], axis=1))
    wef = np.ascontiguousarray(np.asarray(w_expert, np.float32))
    bef = np.asarray(b_expert, np.float32).reshape(1, H)
    has_bias = bool(np.any(bef))
    in_maps = []
    for k in range(NCORES):
        shard = np.ascontiguousarray(xb[k::NCORES].reshape(T_LOC, H))
        sh_h = shard.astype(np.float16)
        sh_l = (shard - sh_h.astype(np.float32)).astype(np.float16)
        ccold = cold.copy()
        # maskk [512, 64] -> [128, 4, 64]
        ccold[:, CC_MASKK:CC_MASKK + 4 * NBLK] = make_maskk(k).reshape(
            4, P, NBLK).transpose(1, 0, 2).reshape(P, 4 * NBLK)
        m = {"x": shard,
             "xth": np.ascontiguousarray(sh_h.T),
             "xtl": np.ascontiguousarray(sh_l.T),
             "wgcat": wgcat, "w_expert": wef,
             "ch": ch, "cc": ccold}
        if has_bias:
            m["b_expert"] = np.ascontiguousarray(bef)
        in_maps.append(m)
    return in_maps


def assemble_out(results, batch_shape):
    T = NCORES * T_LOC
    outf = np.empty((T // BLK, BLK, H), np.float32)
    for k in range(NCORES):
        outf[k::NCORES] = results[k]["out"][:T_LOC].reshape(-1, BLK, H)
    return outf.reshape(batch_shape)


_NC = None
_NC_BIAS = None
LAST_EXEC_NS = None


def _maybe_register_ntff_hook():
    """Best-effort registration of the axon NTFF profiling hook (used only
    when BASS_TRACE is set); harmless if unavailable."""
    try:
        import antenv
        from trn_agent_boot.trn_boot import _ntff_profile_via_ctypes
        if "antenv.axon_hooks" in sys.modules:
            return
        hook = _ntff_profile_via_ctypes("/opt/axon/libaxon_pjrt.so")
        mod = types.ModuleType("antenv.axon_hooks")
        mod.get_axon_ntff_profile_hook = lambda: hook
        mod.set_axon_ntff_profile_hook = lambda h: None
        antenv.axon_hooks = mod
        sys.modules["antenv.axon_hooks"] = mod
        bass_utils.upload_artifacts = lambda tmpdir: f"file://{tmpdir}"
    except Exception:
        pass


def kernel(x, w_gate, w_expert, b_expert):
    global _NC, _NC_BIAS, LAST_EXEC_NS
    if os.environ.get("BASS_TRACE"):
        _maybe_register_ntff_hook()
    in_maps = make_in_maps(x, w_gate, w_expert, b_expert)
    has_bias = "b_expert" in in_maps[0]
    if has_bias:
        if _NC_BIAS is None:
            _NC_BIAS = build(has_bias=True)
        prog = _NC_BIAS
    else:
        if _NC is None:
            _NC = build(has_bias=False)
        prog = _NC
    # The fleet occasionally throws a transient NRT_EXEC_UNIT_UNRECOVERABLE
    # on execute (observed ~10% of invocations; always recovers on retry).
    last_exc = None
    for attempt in range(3):
        try:
            res = bass_utils.run_bass_kernel_spmd(
                prog, in_maps, core_ids=list(range(NCORES)))
            break
        except Exception as exc:
            last_exc = exc
            import time as _time
            _time.sleep(2.0)
    else:
        raise last_exc
    LAST_EXEC_NS = res.exec_time_ns
    return assemble_out(res.results, np.asarray(x).shape)


# revision 28
# speedup vs baseline: 1.1648x; 1.1634x over previous
"""Distributed sparse-MoE routing kernel for 8 Trainium2 NeuronCores (v2).

Algorithm notes
---------------
The reference routes T=16384 tokens (top-1 of E=8 experts, capacity C=100,
tokens past capacity dropped) and applies ONE shared expert weight (H -> H
Linear).  Because the expert weight is shared, the output collapses to

    out[t] = gate_t * (x_t @ W + b)   if token t wins a capacity slot
           = 0                        otherwise

so only <= E*C = 800 of 16384 tokens need the big matmul.  Token t (choosing
expert e) wins a slot iff fewer than C earlier tokens (global order) chose e.

Distribution: tokens sharded over 8 cores in 32-token blocks, strided (core k
owns blocks b with b % 8 == k).  Each core computes per-block expert counts
locally; one small AllGather (64x8 per core) shares them; small matmuls turn
the gathered table into per-block capacity bases.  Everything else is local.

v2 changes vs the 125-138us v1 (trace-driven):
- Router runs as an fp16 hi/lo split instead of fp32: x = x_h + x_l and
  w_gate = w_h + w_l (fp16 pairs, exact decomposition).  Stationary is
  [w_h | w_l] packed [128,16]; streaming x_h then x_l accumulates all four
  cross terms in one PSUM, one vector add folds the halves.  fp16 streams at
  1 cycle/row vs fp32's 4, so the router drops ~30us -> ~8us of PE time.
  Measured offline on the seed-0 data: max logit error 2.9e-6 vs min top-2
  gap 1.39e-5 (16x margin), zero argmax flips.
- Softmax/masks batched over all 16 token tiles in one pass (was 4 groups).
- Post-AG compaction batched: one-shot keep/kf/s, one [128,16] pos matmul
  pair, 16 back-to-back M-builds then 16 back-to-back cmpT matmuls (was an
  8us vector<->PE ping-pong).
- Gate scale folded into the expert-output PSUM->SBUF copy (tensor_scalar
  mult), so the gathered x rows go straight to PE transposes.
- Scatter pads route to a dummy DRAM row (out has 2049 rows; pad slots
  accumulate idx 0 and add 2048) so no bounds-check path is needed.
- Constants consolidated into two blob DMAs (hot: ident+esum needed during
  the router; cold: everything post-AG) instead of ~12 tiny DMAs at ~0.6us
  of sync-engine trigger time each.

Measured constraints on this fleet (do not re-derive):
- A collective costs ~60-85us in an empty kernel; in v1 the AG trigger at
  ~54us completed at ~82us (15.8us peer/launch skew + 12us mesh).
- The PE duty-cycles to 50% (HAM k=4/n=8) after ~30us of sustained fp32
  work; lighter fp16 router reduces this pressure.
- DMA: 16 HW queues, ~22GB/s each when all contend (358GB/s/core cap);
  each dma_start costs ~0.61us of sync-engine trigger time, so batch
  small transfers but keep big ones spread across queues.
- exec_time is core 0's span (only core 0 is profiled by default).
"""
import os
import sys
import types
from contextlib import ExitStack

sys.path.insert(0, "/opt/trn_rl_repo")

import numpy as np

import concourse.bass as bass
import concourse.bacc as bacc
import concourse.mybir as mybir
import concourse.tile as tile
from concourse import bass_utils

F32 = mybir.dt.float32
F32R = mybir.dt.float32r
F16 = mybir.dt.float16
I32 = mybir.dt.int32
AX = mybir.AxisListType
ALU = mybir.AluOpType
ACT = mybir.ActivationFunctionType

P = 128          # SBUF partitions / tile rows
H = 1024         # hidden dim
E = 8            # experts
C = 100          # capacity
NCORES = 8
T_LOC = 2048     # tokens per core
NTILE = T_LOC // P   # 16 token tiles per core
NCH = H // P         # 8 hidden chunks
BLK = 32             # token block size for sharding
NBLK = T_LOC // BLK  # 64 local blocks per core
KMAX = 128           # max compacted (kept) tokens per core (<=114 actual)
TRASH = T_LOC        # dummy out row for pad slots

# cold consts blob column offsets
CC_TRI = 0
CC_IOTA = 128
CC_TIDX = 256
CC_MISC = 272        # rows 0-3 eexp (f32 copy; f16 mirror lives in cf16)
CC_ONESC = 400
CC_NEG1 = 401
CC_H64 = 409
CC_MASKK = 473
CC_ONES1 = 473 + 4 * NBLK    # 729, rows 0-15 all-ones [16, 128]
CC_LT16 = CC_ONES1 + 128     # 857, rows 0-15: LT16[k,m] = k<m
CC_DEXP = CC_LT16 + 16       # 873, rows 0-63: Dexp[j, 8i+e] = (j//4==i)
CC_W = CC_DEXP + 128         # 1001


def build(has_bias=False):
    """Build + compile the SPMD program (identical on all 8 cores)."""
    nc = bacc.Bacc("TRN2", target_bir_lowering=False, debug=False,
                   num_devices=NCORES)

    x = nc.dram_tensor("x", [T_LOC, H], F32, kind="ExternalInput")
    xth = nc.dram_tensor("xth", [H, T_LOC], F16, kind="ExternalInput")
    xtl = nc.dram_tensor("xtl", [H, T_LOC], F16, kind="ExternalInput")
    wgcat = nc.dram_tensor("wgcat", [H, 2 * E], F16, kind="ExternalInput")
    we = nc.dram_tensor("w_expert", [H, H], F32R, kind="ExternalInput")
    be = (nc.dram_tensor("b_expert", [1, H], F32R, kind="ExternalInput")
          if has_bias else None)
    ch = nc.dram_tensor("ch", [P, 132], F32, kind="ExternalInput")
    cc = nc.dram_tensor("cc", [P, CC_W], F32, kind="ExternalInput")
    # f16 consts: rows 0-3 cols 0:128 = eexp, rows 0-63 cols 128:132 = Q64
    cf = nc.dram_tensor("cf", [NBLK, 132], F16, kind="ExternalInput")

    out0 = nc.dram_tensor("out0", [T_LOC + 1, H // 2], F32,
                          kind="ExternalOutput")
    out1 = nc.dram_tensor("out1", [T_LOC + 1, H // 2], F32,
                          kind="ExternalOutput")

    with tile.TileContext(nc) as tc:
        _body(nc, tc, x, xth, xtl, wgcat, we, be, ch, cc, cf, out0, out1)

    nc.compile()
    return nc


def _body(nc, tc, x, xth, xtl, wgcat, we, be, ch, cc, cf, out0, out1):
    with ExitStack() as top:
        sb = top.enter_context(tc.tile_pool(name="sb", bufs=1))
        st = top.enter_context(tc.tile_pool(name="st", bufs=4))
        dram = top.enter_context(tc.tile_pool(name="dram", bufs=1, space="DRAM"))

        # ---- DMA schedule -------------------------------------------------
        # Trigger order is program order on the sync engine (~0.61us each).
        # wgcat + hot consts first (router + transposes need them), then the
        # x halves interleaved by group so the router starts on group 0
        # early, then cold consts (first used in the AG stall), then w_expert
        # (first used in phase C).
        wg_sb = sb.tile([P, NCH * 2 * E], F16, tag="wgcat")
        nc.sync.dma_start(wg_sb[:].rearrange("p (c e) -> p c e", c=NCH),
                          wgcat[:, :].rearrange("(c p) e -> p c e", p=P))
        ch_sb = sb.tile([P, 132], F32, tag="ch")
        nc.sync.dma_start(ch_sb[:], ch[:, :])

        xh_sb = sb.tile([P, NCH * T_LOC], F16, tag="xh")
        xl_sb = sb.tile([P, NCH * T_LOC], F16, tag="xl")
        xh3 = xh_sb[:].rearrange("p (c t) -> p c t", c=NCH)
        xl3 = xl_sb[:].rearrange("p (c t) -> p c t", c=NCH)
        cold_done = False
        cc_sb = sb.tile([P, CC_W], F32, tag="cc")
        for g in range(4):
            for src, dst in ((xth, xh3), (xtl, xl3)):
                for q in range(2):
                    nc.sync.dma_start(
                        dst[:, 4 * q:4 * q + 4, g * 512:(g + 1) * 512],
                        src[4 * q * P:(4 * q + 4) * P,
                            g * 512:(g + 1) * 512].rearrange(
                                "(c p) t -> p c t", p=P))
            if g == 1 and not cold_done:
                nc.sync.dma_start(cc_sb[:], cc[:, :])
                cold_done = True
        cf_sb = sb.tile([NBLK, 132], F16, tag="cf")
        nc.sync.dma_start(cf_sb[:], cf[:, :])

        we_sb = sb.tile([P, NCH * H], F32R, tag="we")
        for c in range(NCH):
            nc.sync.dma_start(we_sb[:, c * H:(c + 1) * H],
                              we[c * P:(c + 1) * P, :])
        be_sb = None
        if be is not None:
            be_sb = sb.tile([1, H], F32R, tag="be")
            nc.sync.dma_start(be_sb[:], be[:, :])

        # const views
        ident = ch_sb[:, 0:128]
        esum = ch_sb[:, 128:132]
        tri = cc_sb[:, CC_TRI:CC_TRI + 128]
        iota = cc_sb[:, CC_IOTA:CC_IOTA + 128]
        tidx = cc_sb[:, CC_TIDX:CC_TIDX + NTILE]
        ones1 = cc_sb[0:1, CC_ONES1:CC_ONES1 + 128]
        ones16 = cc_sb[0:16, CC_ONES1:CC_ONES1 + 128]
        lt16 = cc_sb[0:16, CC_LT16:CC_LT16 + 16]
        dexp = cc_sb[0:64, CC_DEXP:CC_DEXP + 128]
        onescol = cc_sb[:, CC_ONESC:CC_ONESC + 1]
        neg1 = cc_sb[0:1, CC_NEG1:CC_NEG1 + E]
        eexp16 = cf_sb[0:4, 0:128]
        q64 = cf_sb[0:64, 128:132]

        # ---- persistent per-token state ----------------------------------
        masks_sb = sb.tile([P, NTILE * E], F32, tag="masks")
        gate_sb = sb.tile([P, NTILE], F32, tag="gate")
        s_sb = sb.tile([P, NTILE], F32, tag="s")
        kf_sb = sb.tile([P, NTILE], F32, tag="kf")
        bc_sb = sb.tile([4, NTILE * E], F32, tag="bc")

        # ================= PHASE A: router + masks + counts ===============
        logits_sb = sb.tile([P, NTILE * E], F32, tag="logits")
        big_sb = sb.tile([P, NTILE * 2 * E], F32, tag="big")
        with ExitStack() as pa:
            plg = pa.enter_context(tc.tile_pool(name="plg", bufs=1, space="PSUM"))
            ptp = pa.enter_context(tc.tile_pool(name="ptp", bufs=2, space="PSUM"))
            psml = pa.enter_context(tc.tile_pool(name="psml", bufs=1, space="PSUM"))

            # router: 4 groups x 16 fp16 matmuls, all accumulated per group
            lgss = []
            for g in range(4):
                lgT = plg.tile([2 * E, 512], F32, space="PSUM", tag=f"lgT{g}")
                for c in range(NCH):
                    nc.tensor.matmul(
                        lgT[:], lhsT=wg_sb[:, c * 2 * E:(c + 1) * 2 * E],
                        rhs=xh3[:, c, g * 512:(g + 1) * 512],
                        start=(c == 0), stop=False)
                    nc.tensor.matmul(
                        lgT[:], lhsT=wg_sb[:, c * 2 * E:(c + 1) * 2 * E],
                        rhs=xl3[:, c, g * 512:(g + 1) * 512],
                        start=False, stop=(c == NCH - 1))
                lgs = st.tile([2 * E, 512], F32, tag="lgs")
                nc.vector.tensor_copy(lgs[:], lgT[:])
                lgss.append(lgs)

            # transpose all 16 tiles back: [16, 128] -> [128, 16], batched 4
            for b in range(4):
                tp4 = ptp.tile([P, 4 * 2 * E], F32, space="PSUM", tag="tp4")
                for j in range(4):
                    i = 4 * b + j
                    nc.tensor.transpose(
                        tp4[:, j * 2 * E:(j + 1) * 2 * E],
                        lgss[i // 4][:, (i % 4) * P:(i % 4 + 1) * P],
                        ident[:2 * E, :2 * E])
                nc.vector.tensor_copy(
                    big_sb[:, b * 4 * 2 * E:(b + 1) * 4 * 2 * E], tp4[:])

            # fold hi/lo halves: logits[:, (i,e)] = big[:, (i, e)] + big[:, (i, 8+e)]
            big3 = big_sb[:].rearrange("p (i e) -> p i e", e=2 * E)
            lg3 = logits_sb[:].rearrange("p (i e) -> p i e", e=E)
            nc.vector.tensor_tensor(lg3, big3[:, :, 0:E], big3[:, :, E:2 * E],
                                    op=ALU.add)

            # batched softmax / first-max mask over [128, 16, 8]
            m16 = st.tile([P, NTILE], F32, tag="m16")
            nc.vector.reduce_max(m16[:], lg3, axis=AX.X)
            m16b = m16[:].rearrange("p (i o) -> p i o", o=1).to_broadcast(
                [P, NTILE, E])
            d128 = st.tile([P, NTILE * E], F32, tag="d128")
            nc.vector.tensor_tensor(
                d128[:].rearrange("p (i e) -> p i e", e=E), lg3, m16b,
                op=ALU.subtract)
            e128 = st.tile([P, NTILE * E], F32, tag="e128")
            nc.scalar.activation(e128[:], d128[:], ACT.Exp)
            z16 = st.tile([P, NTILE], F32, tag="z16")
            nc.vector.reduce_sum(
                z16[:], e128[:].rearrange("p (i e) -> p i e", e=E), axis=AX.X)
            nc.vector.reciprocal(gate_sb[:], z16[:])
            mraw = st.tile([P, NTILE * E], F32, tag="mraw")
            nc.vector.tensor_tensor(
                mraw[:].rearrange("p (i e) -> p i e", e=E), lg3, m16b,
                op=ALU.is_equal)
            c1 = mraw
            for sh in (1, 2, 4):
                c2 = st.tile([P, NTILE * E], F32, tag=f"cc{sh}")
                c1v = c1[:].rearrange("p (i e) -> p i e", e=E)
                c2v = c2[:].rearrange("p (i e) -> p i e", e=E)
                nc.vector.tensor_copy(c2v[:, :, :sh], c1v[:, :, :sh])
                nc.vector.tensor_tensor(c2v[:, :, sh:], c1v[:, :, sh:],
                                        c1v[:, :, :E - sh], op=ALU.add)
                c1 = c2
            nc.vector.tensor_scalar(masks_sb[:], c1[:], 1.0, None,
                                    op0=ALU.is_equal)
            nc.vector.tensor_tensor(masks_sb[:], masks_sb[:], mraw[:],
                                    op=ALU.mult)

            # per-block expert counts, one matmul
            bcp = psml.tile([4, NTILE * E], F32, space="PSUM", tag="bcp")
            nc.tensor.matmul(bcp[:], lhsT=esum, rhs=masks_sb[:],
                             start=True, stop=True)
            nc.vector.tensor_copy(bc_sb[:], bcp[:])

        # ================= AllGather of per-block counts ==================
        agin = dram.tile([NBLK, E], F32, tag="agin")
        agout = dram.tile([NCORES * NBLK, E], F32, tag="agout")
        nc.sync.dma_start(agin[:].rearrange("(i q) e -> q i e", q=4),
                          bc_sb[:].rearrange("p (i e) -> p i e", e=E))
        # own-counts readback runs before/during the collective
        bc64_sb = sb.tile([NBLK, E], F32, tag="bc64")
        nc.sync.dma_start(bc64_sb[:], agin[:])
        nc.gpsimd.collective_compute(
            "AllGather", ALU.bypass,
            replica_groups=[list(range(NCORES))],
            ins=[agin[:].opt()], outs=[agout[:].opt()])
        agt_sb = sb.tile([P, 4 * E], F32, tag="agt")
        nc.sync.dma_start(agt_sb[:].rearrange("p (c e) -> p c e", c=4),
                          agout[:].rearrange("(c p) e -> p c e", p=P))

        with ExitStack() as pb:
            psml = pb.enter_context(tc.tile_pool(name="psml2", bufs=2, space="PSUM"))
            ploc = pb.enter_context(tc.tile_pool(name="ploc", bufs=1, space="PSUM"))
            pcmp = pb.enter_context(tc.tile_pool(name="pcmp", bufs=1, space="PSUM"))

            # AG-independent matmuls first: they run inside the stall.
            loc4s = []
            for g in range(4):
                loc4 = ploc.tile([P, 32], F32, space="PSUM", tag=f"loc{g}")
                nc.tensor.matmul(loc4[:], lhsT=tri,
                                 rhs=masks_sb[:, 32 * g:32 * (g + 1)],
                                 start=True, stop=False)
                loc4s.append(loc4)
            ab = psml.tile([NBLK, E], F32, space="PSUM", tag="sm")
            nc.tensor.matmul(ab[:], lhsT=cc_sb[0:64, CC_H64:CC_H64 + NBLK],
                             rhs=bc64_sb[:], start=True, stop=False)
            nc.tensor.matmul(ab[:], lhsT=ones1[:, :NBLK], rhs=neg1,
                             start=False, stop=False)
            for c in range(4):
                nc.tensor.matmul(
                    ab[:],
                    lhsT=cc_sb[:, CC_MASKK + c * NBLK:CC_MASKK + (c + 1) * NBLK],
                    rhs=agt_sb[:, c * E:(c + 1) * E],
                    start=False, stop=(c == 3))
            ab_sb = sb.tile([NBLK, E], F32, tag="ab64")
            nc.vector.tensor_copy(ab_sb[:], ab[:])
            # repack [64, 8] (j, e) -> [4, 128] (q, (i, e)) on PE, no DRAM
            ab_exp = st.tile([NBLK, NTILE * E], F16, tag="abexp")
            ab_bc = ab_sb[:].rearrange("p (o e) -> p o e", o=1).to_broadcast(
                [NBLK, NTILE, E])
            nc.vector.tensor_tensor(
                ab_exp[:].rearrange("p (i e) -> p i e", e=E),
                dexp.rearrange("p (i e) -> p i e", e=E), ab_bc, op=ALU.mult)
            adp = psml.tile([4, NTILE * E], F32, space="PSUM", tag="sm")
            nc.tensor.matmul(adp[:], lhsT=q64, rhs=ab_exp[:],
                             start=True, stop=True)
            addbase_sb = sb.tile([4, NTILE * E], F16, tag="addbase")
            nc.vector.tensor_copy(addbase_sb[:], adp[:])

            # ============== PHASE B: keep / gate-scale / compaction =======
            keep_sb = sb.tile([P, NTILE * E], F32, tag="keep")
            for g in range(4):
                loc4 = loc4s[g]
                nc.tensor.matmul(loc4[:], lhsT=eexp16,
                                 rhs=addbase_sb[:, 32 * g:32 * (g + 1)],
                                 start=False, stop=True)
                nc.vector.tensor_scalar(keep_sb[:, 32 * g:32 * (g + 1)],
                                        loc4[:], float(C), None, op0=ALU.is_lt)
            nc.vector.tensor_tensor(keep_sb[:], keep_sb[:], masks_sb[:],
                                    op=ALU.mult)
            keep3 = keep_sb[:].rearrange("p (i e) -> p i e", e=E)
            nc.vector.reduce_sum(kf_sb[:], keep3, axis=AX.X)
            g16b = gate_sb[:].rearrange("p (i o) -> p i o", o=1).to_broadcast(
                [P, NTILE, E])
            s128 = st.tile([P, NTILE * E], F32, tag="s128")
            nc.vector.tensor_tensor(
                s128[:].rearrange("p (i e) -> p i e", e=E), keep3, g16b,
                op=ALU.mult)
            nc.vector.reduce_sum(
                s_sb[:], s128[:].rearrange("p (i e) -> p i e", e=E), axis=AX.X)
            # per-tile kept counts, transposed: tksT[i] = sum_p kf[p, i]
            tksT = psml.tile([NTILE, 1], F32, space="PSUM", tag="sm")
            nc.tensor.matmul(tksT[:], lhsT=kf_sb[:], rhs=onescol,
                             start=True, stop=True)
            tksT_sb = sb.tile([NTILE, 1], F32, tag="tksT")
            nc.vector.tensor_copy(tksT_sb[:], tksT[:])
            # exclusive prefix - 1 via matmul: posbT[i] = sum_{i'<i} tks[i'] - 1
            posbT = psml.tile([NTILE, 1], F32, space="PSUM", tag="sm")
            nc.tensor.matmul(posbT[:], lhsT=lt16, rhs=tksT_sb[:],
                             start=True, stop=False)
            nc.tensor.matmul(posbT[:], lhsT=ones1[:, :NTILE],
                             rhs=neg1[:, 0:1], start=False, stop=True)
            posbT_sb = sb.tile([NTILE, 1], F32, tag="posbT")
            nc.vector.tensor_copy(posbT_sb[:], posbT[:])
            diag16 = st.tile([16, 16], F32, tag="diag16")
            nc.vector.tensor_scalar(diag16[:], ident[:16, :16],
                                    posbT_sb[:, 0:1], None, op0=ALU.mult)

            # slot position per token: within-tile rank + tile base
            pos16 = psml.tile([P, NTILE], F32, space="PSUM", tag="sm")
            nc.tensor.matmul(pos16[:], lhsT=tri, rhs=kf_sb[:],
                             start=True, stop=False)
            nc.tensor.matmul(pos16[:], lhsT=ones16, rhs=diag16[:],
                             start=False, stop=True)
            notk = st.tile([P, NTILE], F32, tag="notk")
            nc.vector.tensor_scalar(notk[:], kf_sb[:], 0.5, 4096.0,
                                    op0=ALU.is_lt, op1=ALU.mult)
            poss = st.tile([P, NTILE], F32, tag="poss")
            nc.vector.tensor_tensor(poss[:], pos16[:], notk[:], op=ALU.add)

            # value pairs [token-idx ; gate-scale] per tile (f16: idx <= 2047
            # exact, gate rounds 2^-11 which is far inside tolerance)
            tsv_sb = sb.tile([P, 2 * NTILE], F16, tag="tsv")
            tsv3 = tsv_sb[:].rearrange("p (i j) -> p i j", j=2)
            nc.vector.tensor_copy(
                tsv3[:, :, 0:1], tidx.rearrange("p (i o) -> p i o", o=1))
            nc.vector.tensor_copy(
                tsv3[:, :, 1:2], s_sb[:].rearrange("p (i o) -> p i o", o=1))

            # one-hot M for all tiles (16 back-to-back vector ops), then the
            # 16 compaction matmuls back-to-back (f16: 1 cyc/row, no fp32
            # double-pump)
            M_all = sb.tile([P, NTILE * KMAX], F16, tag="Mall")
            for i in range(NTILE):
                nc.vector.tensor_scalar(M_all[:, i * KMAX:(i + 1) * KMAX],
                                        iota, poss[:, i:i + 1], None,
                                        op0=ALU.is_equal)
            cmpT = pcmp.tile([2, KMAX], F32, space="PSUM", tag="cmpT")
            for i in range(NTILE):
                nc.tensor.matmul(cmpT[:], lhsT=tsv_sb[:, 2 * i:2 * i + 2],
                                 rhs=M_all[:, i * KMAX:(i + 1) * KMAX],
                                 start=(i == 0), stop=(i == NTILE - 1))

            # extract: transpose [2,128] -> [128,2] on the PE
            cmpT_sb = sb.tile([2, KMAX], F32, tag="cmpTsb")
            nc.vector.tensor_copy(cmpT_sb[:], cmpT[:])
            gst = psml.tile([P, 2], F32, space="PSUM", tag="sm")
            nc.tensor.transpose(gst[:], cmpT_sb[:], ident[:2, :2])
            gs_sb = sb.tile([P, 2], F32, tag="gs")   # col 0 = idx, 1 = s
            nc.vector.tensor_copy(gs_sb[:], gst[:])
            scmp = gs_sb[:, 1:2]
            gidx = sb.tile([P, 1], I32, tag="gidx")
            nc.vector.tensor_copy(gidx[:], gs_sb[:, 0:1])
            # pad slots have idx 0 and s 0 -> route them to the trash row
            padf = st.tile([P, 1], F32, tag="padf")
            nc.vector.tensor_scalar(padf[:], scmp, 0.0, float(TRASH),
                                    op0=ALU.is_equal, op1=ALU.mult)
            gsf = st.tile([P, 1], F32, tag="gsf")
            nc.vector.tensor_tensor(gsf[:], gs_sb[:, 0:1], padf[:],
                                    op=ALU.add)
            sidx = sb.tile([P, 1], I32, tag="sidx")
            nc.vector.tensor_copy(sidx[:], gsf[:])

        # ============== PHASE C: gather, expert matmul, scatter ===========
        with ExitStack() as pc:
            ptp = pc.enter_context(tc.tile_pool(name="ptp2", bufs=2,
                                                space="PSUM"))
            ppo = pc.enter_context(tc.tile_pool(name="ppo", bufs=2,
                                                space="PSUM"))
            xg = st.tile([P, H], F32, tag="xg")
            nc.gpsimd.indirect_dma_start(
                out=xg[:], out_offset=None, in_=x[:, :],
                in_offset=bass.IndirectOffsetOnAxis(ap=gidx[:, :1], axis=0))
            xgT = st.tile([P, H], F32R, tag="xgT")
            for g2 in range(2):
                tp = ptp.tile([P, 512], F32, space="PSUM", tag="tp2")
                for c4 in range(4):
                    c = g2 * 4 + c4
                    nc.tensor.transpose(tp[:, c4 * P:(c4 + 1) * P],
                                        xg[:, c * P:(c + 1) * P], ident)
                nc.vector.tensor_copy(xgT[:, g2 * 512:(g2 + 1) * 512], tp[:])

            for n, outn in enumerate((out0, out1)):
                po = ppo.tile([P, 512], F32, space="PSUM", tag="po")
                for c in range(NCH):
                    nc.tensor.matmul(
                        po[:], lhsT=xgT[:, c * P:(c + 1) * P],
                        rhs=we_sb[:, c * H + n * 512: c * H + (n + 1) * 512],
                        start=(c == 0), stop=(be_sb is None and c == NCH - 1))
                if be_sb is not None:
                    nc.tensor.matmul(po[:], lhsT=ones1.bitcast(F32R),
                                     rhs=be_sb[0:1, n * 512:(n + 1) * 512],
                                     start=False, stop=True)
                # gate scale folded into the PSUM->SBUF copy; scatter each
                # column half as soon as it is ready
                outsb = st.tile([P, 512], F32, tag=f"outsb{n}")
                nc.vector.tensor_scalar(outsb[:], po[:], scmp[:, :1], None,
                                        op0=ALU.mult)
                nc.gpsimd.indirect_dma_start(
                    out=outn[:, :],
                    out_offset=bass.IndirectOffsetOnAxis(ap=sidx[:, :1],
                                                         axis=0),
                    in_=outsb[:], in_offset=None)


# ---------------------------------------------------------------------------
# host side
# ---------------------------------------------------------------------------

def make_consts():
    ident = np.eye(P, dtype=np.float32)
    blk_of = np.arange(P) // BLK
    esum = (blk_of[:, None] == np.arange(4)[None, :]).astype(np.float32)
    ch = np.concatenate([ident, esum], axis=1)           # [128, 132]

    cold = np.zeros((P, CC_W), np.float32)
    cold[:, CC_TRI:CC_TRI + 128] = np.triu(np.ones((P, P), np.float32))
    cold[:, CC_IOTA:CC_IOTA + 128] = np.tile(
        np.arange(KMAX, dtype=np.float32)[None, :], (P, 1))
    cold[:, CC_TIDX:CC_TIDX + NTILE] = (
        np.arange(NTILE, dtype=np.float32)[None, :] * P
        + np.arange(P, dtype=np.float32)[:, None])
    cold[0:4, CC_MISC:CC_MISC + 128] = esum.T
    cold[0:16, CC_ONES1:CC_ONES1 + 128] = 1.0
    cold[:, CC_ONESC] = 1.0
    cold[0:1, CC_NEG1:CC_NEG1 + E] = -1.0
    j = np.arange(NBLK)
    cold[0:64, CC_H64:CC_H64 + NBLK] = (
        j[:, None] < 4 * (j[None, :] // 4)).astype(np.float32)
    k16 = np.arange(NTILE)
    cold[0:16, CC_LT16:CC_LT16 + 16] = (
        k16[:, None] < k16[None, :]).astype(np.float32)
    # Dexp[j, 8i+e] = (j//4 == i)
    i16 = np.arange(NTILE)
    cold[0:64, CC_DEXP:CC_DEXP + 128] = np.repeat(
        (j[:, None] // 4 == i16[None, :]).astype(np.float32), E, axis=1)

    cf = np.zeros((NBLK, 132), np.float16)
    cf[0:4, 0:128] = esum.T.astype(np.float16)
    # Q64[j, q] = (j % 4 == q)
    cf[0:64, 128:132] = (
        j[:, None] % 4 == np.arange(4)[None, :]).astype(np.float16)
    return ch, cold, cf


def make_maskk(k):
    # rows (r*64 + jp) = foreign core r's local block jp (global block 8*jp+r)
    # cols j = my local block (global 8*j + k)
    r = np.arange(NCORES)[:, None, None]
    jp = np.arange(NBLK)[None, :, None]
    jm = np.arange(NBLK)[None, None, :]
    m = (r != k) & (8 * jp + r < 8 * jm + k)
    return m.astype(np.float32).reshape(NCORES * NBLK, NBLK)


def make_in_maps(x, w_gate, w_expert, b_expert):
    xf = np.ascontiguousarray(np.asarray(x, np.float32).reshape(-1, H))
    xb = xf.reshape(-1, BLK, H)          # (512, 32, H)
    ch, cold, cf = make_consts()
    wgf = np.asarray(w_gate, np.float32)
    wg_h = wgf.astype(np.float16)
    wg_l = (wgf - wg_h.astype(np.float32)).astype(np.float16)
    wgcat = np.ascontiguousarray(np.concatenate([wg_h, wg_l], axis=1))
    wef = np.ascontiguousarray(np.asarray(w_expert, np.float32))
    bef = np.asarray(b_expert, np.float32).reshape(1, H)
    has_bias = bool(np.any(bef))
    in_maps = []
    for k in range(NCORES):
        shard = np.ascontiguousarray(xb[k::NCORES].reshape(T_LOC, H))
        sh_h = shard.astype(np.float16)
        sh_l = (shard - sh_h.astype(np.float32)).astype(np.float16)
        ccold = cold.copy()
        # maskk [512, 64] -> [128, 4, 64]
        ccold[:, CC_MASKK:CC_MASKK + 4 * NBLK] = make_maskk(k).reshape(
            4, P, NBLK).transpose(1, 0, 2).reshape(P, 4 * NBLK)
        m = {"x": shard,
             "xth": np.ascontiguousarray(sh_h.T),
             "xtl": np.ascontiguousarray(sh_l.T),
             "wgcat": wgcat, "w_expert": wef,
             "ch": ch, "cc": ccold, "cf": cf}
        if has_bias:
            m["b_expert"] = np.ascontiguousarray(bef)
        in_maps.append(m)
    return in_maps


def assemble_out(results, batch_shape):
    T = NCORES * T_LOC
    outf = np.empty((T // BLK, BLK, H), np.float32)
    for k in range(NCORES):
        full = np.concatenate([results[k]["out0"][:T_LOC],
                               results[k]["out1"][:T_LOC]], axis=1)
        outf[k::NCORES] = full.reshape(-1, BLK, H)
    return outf.reshape(batch_shape)


_NC = None
_NC_BIAS = None
LAST_EXEC_NS = None


def _maybe_register_ntff_hook():
    """Best-effort registration of the axon NTFF profiling hook (used only
    when BASS_TRACE is set); harmless if unavailable."""
    try:
        import antenv
        from trn_agent_boot.trn_boot import _ntff_profile_via_ctypes
        if "antenv.axon_hooks" in sys.modules:
            return
        hook = _ntff_profile_via_ctypes("/opt/axon/libaxon_pjrt.so")
        mod = types.ModuleType("antenv.axon_hooks")
        mod.get_axon_ntff_profile_hook = lambda: hook
        mod.set_axon_ntff_profile_hook = lambda h: None
        antenv.axon_hooks = mod
        sys.modules["antenv.axon_hooks"] = mod
        bass_utils.upload_artifacts = lambda tmpdir: f"file://{tmpdir}"
    except Exception:
        pass


def kernel(x, w_gate, w_expert, b_expert):
    global _NC, _NC_BIAS, LAST_EXEC_NS
    if os.environ.get("BASS_TRACE"):
        _maybe_register_ntff_hook()
    in_maps = make_in_maps(x, w_gate, w_expert, b_expert)
    has_bias = "b_expert" in in_maps[0]
    if has_bias:
        if _NC_BIAS is None:
            _NC_BIAS = build(has_bias=True)
        prog = _NC_BIAS
    else:
        if _NC is None:
            _NC = build(has_bias=False)
        prog = _NC
    # The fleet occasionally throws a transient NRT_EXEC_UNIT_UNRECOVERABLE
    # on execute (observed ~10% of invocations; always recovers on retry).
    last_exc = None
    for attempt in range(3):
        try:
            res = bass_utils.run_bass_kernel_spmd(
                prog, in_maps, core_ids=list(range(NCORES)))
            break
        except Exception as exc:
            last_exc = exc
            import time as _time
            _time.sleep(2.0)
    else:
        raise last_exc
    LAST_EXEC_NS = res.exec_time_ns
    return assemble_out(res.results, np.asarray(x).shape)
